# revision 3
# baseline (speedup 1.0000x reference)
"""MMD loss (RBF kernel, sigma=1) on 8 Trainium2 NeuronCores.

kernel(x, y): x, y float32 [20000, 64] -> float32 scalar
    kxx/nX^2 + kyy/nY^2 - 2*kxy/(nX*nY),  k** = sum_ij exp(-||a_i-b_j||^2/2)

Design (measured ~756us vs the ~929us all-ScalarE baseline)
-----------------------------------------------------------
The whole pair-exponent matrix is produced by ONE fp16 matmul per chunk
(K=69 trick: row vector [a; ha; la; 1; 1] x col vector [b; 1; 1; gb; gl]
gives a.b + s_a + s_b in PSUM, with hi/lo fp16 splits of the squared
norms).  On this part the PE streams 1 column of 128 exponents per
cycle at a fixed 1.2 GHz (the HAM clock gate never releases to 2.4 GHz
in this environment — verified with a 10us continuous-matmul probe), so
the ~8.5e5 streamed columns per core are a ~710us floor.  The
baseline's bottleneck, however, was ScalarE (1 elem/lane/cycle exp on
every pair, ~870us busy).  This kernel splits the elementwise
consumption across BOTH PSUM-capable engines so the reduction rides
under the PE floor:

  - ScalarE chunks: exact exp + accum_out row-sums (as the baseline).
  - VectorE chunks: reduce_max certificate.  If a chunk's max exponent
    is < TAU = -15, its true contribution is < n_elems * e^-15; over
    all V-chunks that bounds the dropped mass by ~1.5e2, far below the
    2e-2 relative-tolerance budget (~8e2 on the raw sums, the value
    being ~2/n).  The host checks every V-chunk max and recomputes any
    flagged chunk exactly in numpy — for Gaussian-like data no chunk
    ever flags (the closest off-diagonal pair sits at m ~ -20).

PSUM layout: 4 tiles of [128, 1024] fp32 (2 banks each) — independent
ping-pong pairs per consumer engine, so ScalarE and VectorE read
different banks concurrently while the PE fills the other two.  Chunks
are emitted column-major per item so the 3-queue input DMA
(sync/gpsimd/scalar DGE) streams each cols tensor strictly ahead of
consumption; the in-tile diagonal coda (exact, fp16) is deferred until
its colsq inputs have landed, and outputs are flushed at the 3/4 mark
to keep the tail short.

kxx/kyy symmetry, ln2 column-doubling, rolled per-core column windows,
and pad-kill rows are inherited from the baseline (see _make_in_maps).
"""

import os

import numpy as np

# problem dims (hardcoded per contract)
N = 20000
D = 64
CORES = 8
BLOCK = N // CORES  # 2500
TILE = 128
N_TILES = 20  # ceil(2500/128)
PAD_BLOCK = TILE * N_TILES  # 2560
KXX_SPAN = 5 * BLOCK  # 12500
K = D + 5  # 69 contraction rows
CHUNK = 1024  # consumer chunk (2 PSUM banks)
MM_N = 512  # matmul moving free dim (1 PSUM bank fp32)
LN2 = float(np.log(2.0))
KILL = np.float16(-30000.0)  # x2 slots -> -60000 -> exp underflows to 0
TAU = -15.0  # V-chunk certificate threshold on the exponent

_CACHE: dict = {}


def _eq_chunks(total, chunk=CHUNK):
    """Equal-width chunks (each <= chunk)."""
    if total <= 0:
        return []
    n = -(-total // chunk)
    base, rem = divmod(total, n)
    out, pos = [], 0
    for i in range(n):
        w = base + (1 if i < rem else 0)
        out.append((pos, w))
        pos += w
    return out


# (cols_name, rw_name, ncols, accumulator index, triangle?)
_ITEMS = [
    ("colsxr", "rwx", KXX_SPAN, 0, True),
    ("colsyr", "rwy", KXX_SPAN, 1, True),
    ("colsyf", "rwx", N, 2, False),
]

# coda chunk packing: (first square, n squares) per chunk, per side
_CODA_GROUPS = [(0, 8), (8, 8), (16, 4)]


def _schedule():
    """Slot descriptors in program order; deterministic, shared by the
    bass builder and the host reducer.  eng 'A' = ScalarE exact exp,
    'V' = VectorE max certificate.  The coda (needs colsqx/colsqy) is
    deferred to slot position CODA_AT so its input DMA can trail the
    main items' lead pieces."""
    CODA_AT = 64
    coda = []
    for side in range(2):  # 0: x squares (kxx acc), 1: y squares (kyy acc)
        for sq0, nsq in _CODA_GROUPS:
            coda.append(
                {"kind": "coda", "eng": "A", "acc": side, "side": side,
                 "sq0": sq0, "nsq": nsq, "cn": TILE * nsq}
            )
    raw = []
    for item_idx, (_cols, _rw, ncols, acc, tri) in enumerate(_ITEMS):
        for r in range(N_TILES):
            base = TILE * (r + 1) if tri else 0
            for c0r, cn in _eq_chunks(ncols - base):
                raw.append(
                    {"kind": "main", "item": item_idx, "acc": acc,
                     "r": r, "c0": base + c0r, "cn": cn}
                )
    # column-major order: consumption walks each cols tensor left-to-right,
    # so the input DMA can stream strictly ahead of the PE.
    raw.sort(key=lambda s: (s["item"], s["c0"], s["r"]))
    ta = tv = 0.0
    for s in raw:
        cn = s["cn"]
        ca = (cn + 166.0) / 1.2 + 363.0
        cv = (cn + 120.0) / 0.96 + 65.0
        if ta + ca <= tv + cv:
            s["eng"], ta = "A", ta + ca
        else:
            s["eng"], tv = "V", tv + cv
    # x-squares after the DMA lead settles; y-squares once colsqy/rwy land
    return (raw[:CODA_AT] + coda[:3] + raw[CODA_AT:110]
            + coda[3:] + raw[110:])


def _build_nc():
    import concourse.bacc as bacc
    import concourse.tile as tile
    from concourse import mybir

    slots = _schedule()
    n_a = sum(1 for s in slots if s["eng"] == "A")
    n_v = len(slots) - n_a

    nc = bacc.Bacc("TRN2", target_bir_lowering=False)
    f16 = mybir.dt.float16
    f32 = mybir.dt.float32
    EXP = mybir.ActivationFunctionType.Exp
    AXX = mybir.AxisListType.X

    dram = {
        "colsxr": nc.dram_tensor("colsxr", [K, KXX_SPAN], f16, kind="ExternalInput"),
        "colsyr": nc.dram_tensor("colsyr", [K, KXX_SPAN], f16, kind="ExternalInput"),
        "colsyf": nc.dram_tensor("colsyf", [K, N], f16, kind="ExternalInput"),
        "colsqx": nc.dram_tensor("colsqx", [K, PAD_BLOCK], f16, kind="ExternalInput"),
        "colsqy": nc.dram_tensor("colsqy", [K, PAD_BLOCK], f16, kind="ExternalInput"),
        "rwx": nc.dram_tensor("rwx", [K, PAD_BLOCK], f16, kind="ExternalInput"),
        "rwy": nc.dram_tensor("rwy", [K, PAD_BLOCK], f16, kind="ExternalInput"),
    }
    parts_d = nc.dram_tensor("parts", [TILE, n_a], f32, kind="ExternalOutput")
    maxes_d = nc.dram_tensor("maxes", [TILE, n_v], f32, kind="ExternalOutput")

    with tile.TileContext(nc) as tc:
        with (
            tc.tile_pool(name="sb", bufs=1) as sb,
            tc.tile_pool(name="psa", bufs=2, space="PSUM") as psa,
            tc.tile_pool(name="psv", bufs=2, space="PSUM") as psv,
        ):
            colsxr = sb.tile([K, KXX_SPAN], f16)
            colsyr = sb.tile([K, KXX_SPAN], f16)
            colsyf = sb.tile([K, N], f16)
            colsqx = sb.tile([K, PAD_BLOCK], f16)
            colsqy = sb.tile([K, PAD_BLOCK], f16)
            rwx = sb.tile([K, PAD_BLOCK], f16)
            rwy = sb.tile([K, PAD_BLOCK], f16)
            parts = sb.tile([TILE, n_a], f32)
            maxes = sb.tile([TILE, n_v], f32)
            zeros = sb.tile([TILE, 1], f32)
            nc.vector.memset(zeros, 0.0)
            sbuf = {
                "colsxr": colsxr, "colsyr": colsyr, "colsyf": colsyf,
                "colsqx": colsqx, "colsqy": colsqy, "rwx": rwx, "rwy": rwy,
            }
            # Priority-ordered DMA over the three DGE-capable queues
            # (sync/gpsimd/scalar).  Lead pieces unblock the first main
            # chunks (~item0 needs rwx[:,:128] and colsxr[:,128:1152])
            # within a few us; everything else streams behind in the order
            # slots will consume it (colsxr rest -> colsq+rwy for the
            # deferred coda -> colsyr -> colsyf).
            # Explicit per-queue DMA plans.  q0 (sync) carries the row
            # tensors + coda columns; q1/q2 (gpsimd/scalar) stream the
            # column tensors left-to-right, alternating blocks, matching
            # the column-major slot order.  Cols [0:128) of the rolled
            # windows are never read (triangle base >= 128) and are skipped.
            q_plan = {
                0: [("rwx", 0, 384), ("rwx", 384, 1024), ("rwx", 1408, 1152),
                    ("rwy", 0, 2560), ("colsyr", 128, 2432),
                    ("colsyf", 16000, 4000)],
                1: [("colsxr", 128, 384), ("colsxr", 512, 640),
                    ("colsxr", 1152, 1024), ("colsqx", 0, 2560),
                    ("colsxr", 3200, 1536), ("colsxr", 6272, 1536),
                    ("colsxr", 9344, 1536),
                    ("colsyr", 2560, 2560), ("colsyr", 7680, 2560),
                    ("colsyf", 0, 4000), ("colsyf", 8000, 4000)],
                2: [("colsxr", 2176, 1024), ("colsqy", 0, 2560),
                    ("colsxr", 4736, 1536), ("colsxr", 7808, 1536),
                    ("colsxr", 10880, 1620),
                    ("colsyr", 5120, 2560), ("colsyr", 10240, 2260),
                    ("colsyf", 4000, 4000), ("colsyf", 12000, 4000)],
            }
            dma_engines = {0: nc.sync, 1: nc.gpsimd, 2: nc.scalar}
            for q, plan in q_plan.items():
                for name, p0, w in plan:
                    dma_engines[q].dma_start(
                        out=sbuf[name][:, p0 : p0 + w],
                        in_=dram[name][:, p0 : p0 + w],
                    )

            ia = iv = 0
            flushed_a = flushed_v = 0
            last_w = None  # stationary-operand key of the previous matmul;
            # repeat matmuls skip the redundant LDWEIGHTS (self-load off)
            for si, s in enumerate(slots):
                if si == (3 * len(slots)) // 4:
                    # flush the output columns finished so far; only the
                    # remainder rides in the kernel tail
                    nc.sync.dma_start(out=parts_d[:, :ia], in_=parts[:, :ia])
                    nc.gpsimd.dma_start(out=maxes_d[:, :iv], in_=maxes[:, :iv])
                    flushed_a, flushed_v = ia, iv
                if s["eng"] == "A":
                    pt = psa.tile([TILE, CHUNK], f32, tag="pa", name=f"pa{si}")
                else:
                    pt = psv.tile([TILE, CHUNK], f32, tag="pv", name=f"pv{si}")
                cn = s["cn"]
                if s["kind"] == "coda":
                    rw, colsq = ((rwx, colsqx), (rwy, colsqy))[s["side"]]
                    for k in range(s["nsq"]):
                        sq = s["sq0"] + k
                        sl = slice(TILE * sq, TILE * (sq + 1))
                        nc.tensor.matmul(
                            pt[:, TILE * k : TILE * (k + 1)],
                            rw[:, sl],
                            colsq[:, sl],
                            start=True,
                            stop=True,
                        )
                        last_w = ("coda", s["side"], sq)
                else:
                    cols_name, rw_name, _ncols, _acc, _tri = _ITEMS[s["item"]]
                    cols, rw = sbuf[cols_name], sbuf[rw_name]
                    lhsT = rw[:, s["r"] * TILE : (s["r"] + 1) * TILE]
                    c0 = s["c0"]
                    wkey = ("main", s["item"], s["r"])
                    for s0 in range(0, cn, MM_N):
                        sn = min(MM_N, cn - s0)
                        inst = nc.tensor.matmul(
                            pt[:, s0 : s0 + sn],
                            lhsT,
                            cols[:, c0 + s0 : c0 + s0 + sn],
                            start=True,
                            stop=True,
                        )
                        if wkey == last_w:
                            inst.ins.ldweights = False
                        last_w = wkey
                if s["eng"] == "A":
                    nc.scalar.activation(
                        out=pt[:, :cn],
                        in_=pt[:, :cn],
                        func=EXP,
                        bias=zeros[:, 0:1],
                        scale=1.0,
                        accum_out=parts[:, ia : ia + 1],
                    )
                    ia += 1
                else:
                    nc.vector.reduce_max(
                        out=maxes[:, iv : iv + 1],
                        in_=pt[:, :cn],
                        axis=AXX,
                    )
                    iv += 1
            nc.sync.dma_start(out=parts_d[:, flushed_a:], in_=parts[:, flushed_a:])
            nc.gpsimd.dma_start(out=maxes_d[:, flushed_v:], in_=maxes[:, flushed_v:])
    nc.compile()
    return nc


def _prep_side(v):
    """v [N, D] fp32 -> (vh fp16 [N, D], s fp64 [N] = -|vh|^2/2)"""
    vh = v.astype(np.float16)
    s = -0.5 * np.sum(vh.astype(np.float64) ** 2, axis=1)
    return vh, s


def _hilo(s):
    h = s.astype(np.float16)
    l = (s - h.astype(np.float64)).astype(np.float16)
    return h, l


def _cols_tensor(vh, g):
    """[K, n] fp16 column tensor: [b; 1; 1; gh; gl]."""
    n = vh.shape[0]
    out = np.zeros((K, n), dtype=np.float16)
    out[:D] = vh.T
    out[D] = 1.0
    out[D + 1] = 1.0
    out[D + 2], out[D + 3] = _hilo(g)
    return np.ascontiguousarray(out)


def _rw_tensor(vh_block, s_block):
    """[K, PAD_BLOCK] fp16 row tensor: [a; ha; la; 1; 1]; pad rows killed."""
    n = vh_block.shape[0]
    rw = np.zeros((K, PAD_BLOCK), dtype=np.float16)
    rw[:D, :n] = vh_block.T
    rw[D, :n], rw[D + 1, :n] = _hilo(s_block)
    rw[D, n:] = KILL  # pad rows: ha * 1 = -30000 -> exp -> 0
    rw[D + 2, :n] = 1.0
    rw[D + 3, :n] = 1.0
    return rw


def _colsq_tensor(vh_block, s_block):
    """Coda columns: own block padded to PAD_BLOCK, pad cols killed."""
    n = vh_block.shape[0]
    vh_pad = np.zeros((PAD_BLOCK, D), dtype=np.float16)
    vh_pad[:n] = vh_block
    g = np.full(PAD_BLOCK, float(KILL), dtype=np.float64)
    g[:n] = s_block
    return _cols_tensor(vh_pad, g)


def _make_in_maps(x, y):
    xh, sx = _prep_side(x)
    yh, sy = _prep_side(y)
    colsyf = _cols_tensor(yh, sy)
    w2 = np.zeros(KXX_SPAN)
    w2[: 4 * BLOCK] = LN2  # diag-block uppers + distance 1..3: doubled

    in_maps = []
    for c in range(CORES):
        order = (np.arange(KXX_SPAN) + BLOCK * c) % N
        blk = slice(BLOCK * c, BLOCK * (c + 1))
        in_maps.append(
            {
                "colsxr": _cols_tensor(xh[order], sx[order] + w2),
                "colsyr": _cols_tensor(yh[order], sy[order] + w2),
                "colsyf": colsyf,
                "colsqx": _colsq_tensor(xh[blk], sx[blk]),
                "colsqy": _colsq_tensor(yh[blk], sy[blk]),
                "rwx": _rw_tensor(xh[blk], sx[blk]),
                "rwy": _rw_tensor(yh[blk], sy[blk]),
            }
        )
    return in_maps


def _exact_chunk(core, s, x, y):
    """Exact (fp64) contribution of a flagged V-chunk, mirroring the
    device's pair coverage and ln2 doubling."""
    item_idx = s["item"]
    r, c0, cn = s["r"], s["c0"], s["cn"]
    r0 = core * BLOCK + r * TILE
    r1 = min(core * BLOCK + (r + 1) * TILE, (core + 1) * BLOCK)
    if item_idx == 2:  # kxy: x rows vs full y
        A = x[r0:r1].astype(np.float64)
        B = y[c0 : c0 + cn].astype(np.float64)
        w = np.ones(cn)
    else:
        v = x if item_idx == 0 else y
        order = (np.arange(KXX_SPAN) + BLOCK * core) % N
        idx = order[c0 : c0 + cn]
        A = v[r0:r1].astype(np.float64)
        B = v[idx].astype(np.float64)
        w = np.where(np.arange(c0, c0 + cn) < 4 * BLOCK, 2.0, 1.0)
    sq = (
        np.sum(A * A, axis=1)[:, None]
        + np.sum(B * B, axis=1)[None, :]
        - 2.0 * (A @ B.T)
    )
    sq = np.maximum(sq, 0.0)
    return float(np.sum(np.exp(-sq / 2.0) * w[None, :]))


def kernel(x, y):
    from concourse.bass_utils import run_bass_kernel_spmd

    x = np.asarray(x, dtype=np.float32)
    y = np.asarray(y, dtype=np.float32)
    assert x.shape == (N, D) and y.shape == (N, D)

    if "nc" not in _CACHE:
        _CACHE["nc"] = _build_nc()
    nc = _CACHE["nc"]

    in_maps = _make_in_maps(x, y)
    trace = os.environ.get("MMD_TRACE", "0") == "1"
    try:
        br = run_bass_kernel_spmd(
            nc, in_maps, core_ids=list(range(CORES)), trace=trace
        )
    except Exception:
        if not trace:
            raise
        import traceback

        traceback.print_exc()
        print("trace run failed; retrying without trace")
        br = run_bass_kernel_spmd(
            nc, in_maps, core_ids=list(range(CORES)), trace=False
        )
    _CACHE["last_results"] = br

    slots = _schedule()
    tot = np.zeros(3, dtype=np.float64)
    n_flagged = 0
    for core, core_res in enumerate(br.results):
        parts = core_res["parts"].astype(np.float64)
        maxes = core_res["maxes"]
        ia = iv = 0
        for s in slots:
            if s["eng"] == "A":
                tot[s["acc"]] += float(parts[:, ia].sum())
                ia += 1
            else:
                if float(maxes[:, iv].max()) >= TAU:
                    tot[s["acc"]] += _exact_chunk(core, s, x, y)
                    n_flagged += 1
                iv += 1
    if n_flagged:
        print(f"kernel: {n_flagged} V-chunks flagged; host-recomputed exactly")
    val = tot[0] / (N * N) + tot[1] / (N * N) - 2.0 * tot[2] / (N * N)
    return np.array(val, dtype=np.float32)
